# revision 6
# baseline (speedup 1.0000x reference)
"""DFSMN (order-9 IIR + 2-tap lookahead FIR along frames) on 8 Trainium2 cores.

Math: the torch/jax reference computes, per (b, h, d) sequence along frames t:
    p[t] = base[t] + sum_{k=1..9} c_k[d] * p[t-k]
    base[t] = (1 + l0[d]) v[t] + r1[d] v[t+1] + r2[d] v[t+2]
This is a per-channel LTI filter, so p = W_d * v (causal-ish convolution with
the filter's impulse response, which decays like rho^n with rho <= ~0.89).
Lags beyond ~170 are below fp32 resolution, so each 128-frame output block
depends only on the previous 256 input frames. That turns the whole problem
into, per channel d:

    out_block(b) = W1_d^T @ x[window b] + W2_d^T @ x[window b+1]

with W1/W2 128x128 Toeplitz matrices built (on host) from the impulse
response, and windows = consecutive 128-frame chunks of the front-padded
input. A tiny rank-2 matmul corrects block 0 for the "base does not exist
for t<0" boundary (folding the r-taps into W pretends it does).

Sharding: channels d (512) split across 8 cores (64 each); all 64 (b,h)
sequences ride the matmul free dimension. Zero cross-core communication.

Layout per core (host-prepared):
    x     [64 d, 1152 t_padded, 64 bh] f32   (126 zero frames front, 2 back)
    wseq  [64 d, 383]  f32   filter taps w[lag], lag in [-129, 253] (0 below -2)
    corrw [64 d, 2, 128] f32 rank-2 block-0 boundary correction
    y     [64 d, 1024 t, 64 bh] f32

The Toeplitz matrices are materialized SBUF-side by diagonal-strided DMA
reads of wseq (step -1 along partitions, +1 along free), so weight HBM
traffic is ~1.5KB per channel instead of 128KB.
"""

import numpy as np

import concourse.bass as bass
import concourse.bacc as bacc
import concourse.mybir as mybir
from concourse import tile
from concourse import bass_utils

B, H, T, D = 16, 4, 1024, 512
N_CORES = 8
DC = D // N_CORES          # 64 channels per core
BH = B * H                 # 64 sequences (matmul free dim)
NBLK = T // 128            # 8 output blocks
PADF = 126                 # leading zero frames (window 0 history)
TPAD = PADF + T + 2        # 1152 = 9 * 128 exactly
NWIN = TPAD // 128         # 9 windows
NW = 383                   # wseq length: lags -129..253
F32 = mybir.dt.float32

_NC_CACHE: dict = {}


def _build_nc(dc: int = DC):
    nc = bacc.Bacc("TRN2", target_bir_lowering=False, debug=False)
    x = nc.dram_tensor("x", [dc, TPAD, BH], F32, kind="ExternalInput")
    wseq = nc.dram_tensor("wseq", [dc, NW], F32, kind="ExternalInput")
    corrw = nc.dram_tensor("corrw", [dc, 2, 128], F32, kind="ExternalInput")
    y = nc.dram_tensor("y", [dc, T, BH], F32, kind="ExternalOutput")
    xap, wap, cap, yap = x.ap(), wseq.ap(), corrw.ap(), y.ap()

    with tile.TileContext(nc) as tc:
        with tc.tile_pool(name="xp", bufs=4) as xp, \
             tc.tile_pool(name="wp", bufs=4) as wp, \
             tc.tile_pool(name="cp", bufs=4) as cp, \
             tc.tile_pool(name="op", bufs=4) as op, \
             tc.tile_pool(name="pp", bufs=4, space="PSUM") as pp:
            for d in range(dc):
                # input windows: SBUF [128 t-in-window, (win, bh)]
                xt = xp.tile([128, NWIN * BH], F32, name="xt")
                src = xap.copy()
                src.ap = src.ap[:0] + [[BH, 128], [128 * BH, NWIN], [1, BH]]
                src.offset = d * TPAD * BH
                nc.sync.dma_start(out=xt, in_=src)

                # Toeplitz weights via diagonal-strided reads of the
                # REVERSED tap sequence (wrev[p] = wseq[382-p]), so the
                # DMA partition step stays +1 (negative partition steps are
                # rejected by the BIR verifier); the -1 step rides the free
                # dim instead. W1[k,i] = w[i-k+126] = wrev[127+k-i].
                wt = wp.tile([128, 256], F32, name="wt")
                w1 = wap.copy()
                w1.ap = w1.ap[:0] + [[1, 128], [-1, 128]]
                w1.offset = d * NW + 127
                nc.sync.dma_start(out=wt[:, 0:128], in_=w1)
                w2 = wap.copy()
                w2.ap = w2.ap[:0] + [[1, 128], [-1, 128]]
                w2.offset = d * NW + 255
                nc.sync.dma_start(out=wt[:, 128:256], in_=w2)

                # block-0 boundary correction operands
                cw = cp.tile([2, 128], F32, name="cw")
                nc.sync.dma_start(out=cw, in_=cap[d])
                v01 = cp.tile([2, BH], F32, name="v01")
                nc.sync.dma_start(out=v01, in_=xap[d, PADF:PADF + 2, :])

                ps = pp.tile([128, NBLK * BH], F32, name="ps")
                nc.tensor.matmul(ps, lhsT=wt[:, 0:128],
                                 rhs=xt[:, 0:NBLK * BH],
                                 start=True, stop=False)
                nc.tensor.matmul(ps[:, 0:BH], lhsT=cw, rhs=v01,
                                 start=False, stop=False)
                nc.tensor.matmul(ps, lhsT=wt[:, 128:256],
                                 rhs=xt[:, BH:(NBLK + 1) * BH],
                                 start=False, stop=True)

                ot = op.tile([128, NBLK * BH], F32, name="ot")
                if d % 2 == 0:
                    nc.scalar.copy(ot, ps)
                else:
                    nc.vector.tensor_copy(ot, ps)

                dst = yap.copy()
                dst.ap = dst.ap[:0] + [[BH, 128], [128 * BH, NBLK], [1, BH]]
                dst.offset = d * T * BH
                nc.sync.dma_start(out=dst, in_=ot)
    nc.compile()
    return nc


def _get_nc(dc: int = DC):
    if dc not in _NC_CACHE:
        _NC_CACHE[dc] = _build_nc(dc)
    return _NC_CACHE[dc]


def _build_filters(l_filter: np.ndarray, r_filter: np.ndarray):
    """Host-side filter prep: impulse response -> padded tap sequence wseq
    [383, D] (lag l at index l+129) and rank-2 block-0 correction [2, 128, D].
    """
    c = l_filter[1:].astype(np.float64)            # (9, D) IIR coeffs
    d = c.shape[1]
    a = np.zeros((258, d))
    a[0] = 1.0
    for n in range(1, 258):
        for k in range(1, min(9, n) + 1):
            a[n] += c[k - 1] * a[n - k]
    q0 = 1.0 + l_filter[0].astype(np.float64)
    q1 = r_filter[0].astype(np.float64)
    q2 = r_filter[1].astype(np.float64)

    wseq = np.zeros((NW, d))
    for lag in range(-2, 254):
        t = q2 * a[lag + 2]
        if lag + 1 >= 0:
            t = t + q1 * a[lag + 1]
        if lag >= 0:
            t = t + q0 * a[lag]
        wseq[lag + 129] = t

    ii = np.arange(128)
    corrw = np.stack([-(q1[None, :] * a[ii + 1] + q2[None, :] * a[ii + 2]),
                      -(q2[None, :] * a[ii + 1])], axis=0)   # (2, 128, D)
    return wseq.astype(np.float32), corrw.astype(np.float32)


def _make_in_maps(v, l_filter, r_filter, n_cores=N_CORES, dc=DC):
    wseq, corrw = _build_filters(l_filter, r_filter)
    vr = np.asarray(v, dtype=np.float32).reshape(BH, T, D)
    in_maps = []
    for cid in range(n_cores):
        sl = slice(cid * dc, (cid + 1) * dc)
        xcore = np.zeros((dc, TPAD, BH), np.float32)
        xcore[:, PADF:PADF + T, :] = vr[:, :, sl].transpose(2, 1, 0)
        in_maps.append({
            "x": np.ascontiguousarray(xcore),
            "wseq": np.ascontiguousarray(wseq[::-1, sl].T),
            "corrw": np.ascontiguousarray(corrw[:, :, sl].transpose(2, 0, 1)),
        })
    return in_maps


def kernel(v: np.ndarray, l_filter: np.ndarray, r_filter: np.ndarray,
           **_unused) -> np.ndarray:
    nc = _get_nc(DC)
    in_maps = _make_in_maps(v, l_filter, r_filter)
    res = bass_utils.run_bass_kernel_spmd(nc, in_maps, core_ids=list(range(N_CORES)))
    out = np.empty((D, T, BH), np.float32)
    for cid in range(N_CORES):
        out[cid * DC:(cid + 1) * DC] = res.results[cid]["y"]
    return np.ascontiguousarray(out.transpose(2, 1, 0)).reshape(B, H, T, D)


# revision 7
# speedup vs baseline: 48.0882x; 48.0882x over previous
"""DFSMN (order-9 IIR + 2-tap lookahead FIR along frames) on 8 Trainium2 cores.

Math: the reference computes, per (b, h, d) sequence along frames t:
    p[t] = base[t] + sum_{k=1..9} c_k[d] * p[t-k]
    base[t] = (1 + l0[d]) v[t] + r1[d] v[t+1] + r2[d] v[t+2]
This is a per-channel LTI filter, so p = w_d * v (convolution with the
filter's impulse response, which decays like rho^n with rho <= ~0.89, below
fp32 resolution past lag ~170). Each 128-frame output block therefore
depends only on the previous 256 input frames, which turns the whole
problem into, per channel d:

    out_block(b) = W1_d^T @ x[window b] + W2_d^T @ x[window b+1]

with W1/W2 128x128 Toeplitz matrices built on the host from the impulse
response, and windows = consecutive 128-frame chunks of the front-padded
input. Because the window offset advances exactly BH free-elements per
block, all 8 blocks collapse into a single FD=512 matmul per term. A tiny
rank-2 matmul corrects block 0 for the "base does not exist for t<0"
boundary (folding the r-taps into W pretends it does).

Sharding: channels d (512) split across 8 cores (64 each); all 64 (b,h)
sequences ride the matmul free dimension. Zero cross-device communication.

Per-core inputs (host-prepared):
    x    [64 d, 1152 t_padded, 64 bh] f32  (126 zero frames front, 2 back)
    wmat [64 d, 128 k, 256]  f32  concat(W1, W2) Toeplitz matrices
    cv   [64 d, 2, 192]      f32  concat(rank-2 correction [2,128], v[0:2] [2,64])
    y    [64 d, 1024 t, 64 bh] f32
"""

import numpy as np

import concourse.bass as bass
import concourse.bacc as bacc
import concourse.mybir as mybir
from concourse import tile
from concourse import bass_utils

B, H, T, D = 16, 4, 1024, 512
N_CORES = 8
DC = D // N_CORES          # 64 channels per core
BH = B * H                 # 64 sequences (matmul free dim)
NBLK = T // 128            # 8 output blocks
PADF = 126                 # leading zero frames (window 0 history)
TPAD = PADF + T + 2        # 1152 = 9 * 128 exactly
NWIN = TPAD // 128         # 9 windows
F32 = mybir.dt.float32

_NC_CACHE: dict = {}


def _build_nc(dc: int = DC):
    nc = bacc.Bacc("TRN2", target_bir_lowering=False, debug=False)
    x = nc.dram_tensor("x", [dc, TPAD, BH], F32, kind="ExternalInput")
    wmat = nc.dram_tensor("wmat", [dc, 128, 256], F32, kind="ExternalInput")
    cv = nc.dram_tensor("cv", [dc, 2, 192], F32, kind="ExternalInput")
    y = nc.dram_tensor("y", [dc, T, BH], F32, kind="ExternalOutput")
    xap, yap = x.ap(), y.ap()

    with tile.TileContext(nc) as tc:
        with tc.tile_pool(name="xp", bufs=4) as xp, \
             tc.tile_pool(name="wp", bufs=4) as wp, \
             tc.tile_pool(name="cp", bufs=4) as cp, \
             tc.tile_pool(name="op", bufs=4) as op, \
             tc.tile_pool(name="pp", bufs=4, space="PSUM") as pp:
            for d in range(dc):
                # input windows: SBUF [128 t-in-window, (win, bh)]
                xt = xp.tile([128, NWIN * BH], F32, name="xt")
                src = xap.copy()
                src.ap = src.ap[:0] + [[BH, 128], [128 * BH, NWIN], [1, BH]]
                src.offset = d * TPAD * BH
                nc.sync.dma_start(out=xt, in_=src)

                wt = wp.tile([128, 256], F32, name="wt")
                nc.scalar.dma_start(out=wt, in_=wmat.ap()[d])
                cvt = cp.tile([2, 192], F32, name="cvt")
                nc.scalar.dma_start(out=cvt, in_=cv.ap()[d])

                ps = pp.tile([128, NBLK * BH], F32, name="ps")
                nc.tensor.matmul(ps, lhsT=wt[:, 0:128],
                                 rhs=xt[:, 0:NBLK * BH],
                                 start=True, stop=False)
                nc.tensor.matmul(ps[:, 0:BH], lhsT=cvt[:, 0:128],
                                 rhs=cvt[:, 128:192],
                                 start=False, stop=False)
                nc.tensor.matmul(ps, lhsT=wt[:, 128:256],
                                 rhs=xt[:, BH:(NBLK + 1) * BH],
                                 start=False, stop=True)

                ot = op.tile([128, NBLK * BH], F32, name="ot")
                nc.vector.tensor_copy(ot, ps)

                dst = yap.copy()
                dst.ap = dst.ap[:0] + [[BH, 128], [128 * BH, NBLK], [1, BH]]
                dst.offset = d * T * BH
                nc.gpsimd.dma_start(out=dst, in_=ot)
    nc.compile()
    return nc


def _get_nc(dc: int = DC):
    if dc not in _NC_CACHE:
        _NC_CACHE[dc] = _build_nc(dc)
    return _NC_CACHE[dc]


def _build_filters(l_filter: np.ndarray, r_filter: np.ndarray):
    """Host-side filter prep.

    Returns wmat [128, 256, D] (k, i; W1 = [:, :128], W2 = [:, 128:]) and the
    rank-2 block-0 boundary correction corr [2, 128, D].
    """
    c = l_filter[1:].astype(np.float64)            # (9, D) IIR coeffs
    d = c.shape[1]
    a = np.zeros((258, d))
    a[0] = 1.0
    for n in range(1, 258):
        for k in range(1, min(9, n) + 1):
            a[n] += c[k - 1] * a[n - k]
    q0 = 1.0 + l_filter[0].astype(np.float64)
    q1 = r_filter[0].astype(np.float64)
    q2 = r_filter[1].astype(np.float64)

    # wseq[lag + 129] = combined FIR tap at lag, lag in [-129, 253] (0 < -2)
    wseq = np.zeros((383, d))
    for lag in range(-2, 254):
        t = q2 * a[lag + 2]
        if lag + 1 >= 0:
            t = t + q1 * a[lag + 1]
        if lag >= 0:
            t = t + q0 * a[lag]
        wseq[lag + 129] = t

    kk = np.arange(128)[:, None]
    ii = np.arange(128)[None, :]
    w1 = wseq[ii - kk + 255]                       # (128, 128, D)
    w2 = wseq[ii - kk + 127]
    wmat = np.concatenate([w1, w2], axis=1)        # (128, 256, D)

    i1 = np.arange(128)
    corr = np.stack([-(q1[None, :] * a[i1 + 1] + q2[None, :] * a[i1 + 2]),
                     -(q2[None, :] * a[i1 + 1])], axis=0)   # (2, 128, D)
    return wmat.astype(np.float32), corr.astype(np.float32)


def _make_in_maps(v, l_filter, r_filter, n_cores=N_CORES, dc=DC):
    wmat, corr = _build_filters(l_filter, r_filter)
    vr = np.asarray(v, dtype=np.float32).reshape(BH, T, D)
    in_maps = []
    for cid in range(n_cores):
        sl = slice(cid * dc, (cid + 1) * dc)
        xcore = np.zeros((dc, TPAD, BH), np.float32)
        xcore[:, PADF:PADF + T, :] = vr[:, :, sl].transpose(2, 1, 0)
        cvcore = np.zeros((dc, 2, 192), np.float32)
        cvcore[:, :, 0:128] = corr[:, :, sl].transpose(2, 0, 1)
        cvcore[:, :, 128:192] = vr[:, 0:2, sl].transpose(2, 1, 0)
        in_maps.append({
            "x": np.ascontiguousarray(xcore),
            "wmat": np.ascontiguousarray(wmat[:, :, sl].transpose(2, 0, 1)),
            "cv": cvcore,
        })
    return in_maps


def kernel(v: np.ndarray, l_filter: np.ndarray, r_filter: np.ndarray,
           **_unused) -> np.ndarray:
    nc = _get_nc(DC)
    in_maps = _make_in_maps(v, l_filter, r_filter)
    res = bass_utils.run_bass_kernel_spmd(nc, in_maps, core_ids=list(range(N_CORES)))
    out = np.empty((D, T, BH), np.float32)
    for cid in range(N_CORES):
        out[cid * DC:(cid + 1) * DC] = res.results[cid]["y"]
    return np.ascontiguousarray(out.transpose(2, 1, 0)).reshape(B, H, T, D)


# revision 11
# speedup vs baseline: 53.8801x; 1.1204x over previous
"""DFSMN (order-9 IIR + 2-tap lookahead FIR along frames) on 8 Trainium2 cores.

Math: the reference computes, per (b, h, d) sequence along frames t:
    p[t] = base[t] + sum_{k=1..9} c_k[d] * p[t-k]
    base[t] = (1 + l0[d]) v[t] + r1[d] v[t+1] + r2[d] v[t+2]
This is a per-channel LTI filter, so p = w_d * v (convolution with the
filter's impulse response, which decays like rho^n with rho <= ~0.89, below
fp32 resolution past lag ~170). Each 128-frame output block therefore
depends only on the previous 256 input frames, which turns the whole
problem into, per channel d:

    out_block(b) = W1_d^T @ x[window b] + W2_d^T @ x[window b+1]

with W1/W2 128x128 Toeplitz matrices built on the host from the impulse
response, and windows = consecutive 128-frame chunks of the front-padded
input. Because the window offset advances exactly BH free-elements per
block, all 8 blocks collapse into a single FD=512 matmul per term. A tiny
rank-2 matmul corrects block 0 for the "base does not exist for t<0"
boundary (folding the r-taps into W pretends it does).

Sharding: channels d (512) split across 8 cores (64 each); all 64 (b,h)
sequences ride the matmul free dimension. Zero cross-device communication.

Per-core inputs (host-prepared):
    x    [64 d, 1152 t_padded, 64 bh] f32  (126 zero frames front, 2 back)
    wmat [64 d, 128 k, 256]  f32  concat(W1, W2) Toeplitz matrices
    cv   [64 d, 2, 192]      f32  concat(rank-2 correction [2,128], v[0:2] [2,64])
    y    [64 d, 1024 t, 64 bh] f32
"""

import numpy as np

import concourse.bass as bass
import concourse.bacc as bacc
import concourse.mybir as mybir
from concourse import tile
from concourse import bass_utils

B, H, T, D = 16, 4, 1024, 512
N_CORES = 8
DC = D // N_CORES          # 64 channels per core
BH = B * H                 # 64 sequences (matmul free dim)
NBLK = T // 128            # 8 output blocks
PADF = 126                 # leading zero frames (window 0 history)
TPAD = PADF + T + 2        # 1152 = 9 * 128 exactly
NWIN = TPAD // 128         # 9 windows
F32 = mybir.dt.float32

_NC_CACHE: dict = {}

# Big-matmul operand dtype: float32r streams 1 row/cycle at FD>=256 (vs
# fp32's hi/lo pair at ~4-5 cyc/row); numerics differ from true fp32 on HW.
MM_DT = mybir.dt.float32r


def _build_nc(dc: int = DC):
    nc = bacc.Bacc("TRN2", target_bir_lowering=False, debug=False)
    x = nc.dram_tensor("x", [dc, TPAD, BH], F32, kind="ExternalInput")
    wmat = nc.dram_tensor("wmat", [dc, 128, 256], F32, kind="ExternalInput")
    cv = nc.dram_tensor("cv", [dc, 2, 192], F32, kind="ExternalInput")
    y = nc.dram_tensor("y", [dc, T, BH], F32, kind="ExternalOutput")
    xap, yap = x.ap(), y.ap()

    with tile.TileContext(nc) as tc:
        with tc.tile_pool(name="xp", bufs=4) as xp, \
             tc.tile_pool(name="wp", bufs=4) as wp, \
             tc.tile_pool(name="cp", bufs=4) as cp, \
             tc.tile_pool(name="op", bufs=4) as op, \
             tc.tile_pool(name="pp", bufs=4, space="PSUM") as pp:
            for d in range(dc):
                # input windows: SBUF [128 t-in-window, (win, bh)]
                xt = xp.tile([128, NWIN * BH], MM_DT, name="xt")
                src = xap.copy()
                src.ap = src.ap[:0] + [[BH, 128], [128 * BH, NWIN], [1, BH]]
                src.offset = d * TPAD * BH
                nc.sync.dma_start(out=xt, in_=src.bitcast(MM_DT))

                wt = wp.tile([128, 256], MM_DT, name="wt")
                nc.scalar.dma_start(out=wt, in_=wmat.ap()[d].bitcast(MM_DT))
                cvt = cp.tile([2, 192], F32, name="cvt")
                nc.scalar.dma_start(out=cvt, in_=cv.ap()[d])

                ps = pp.tile([128, NBLK * BH], F32, name="ps")
                nc.tensor.matmul(ps, lhsT=wt[:, 0:128],
                                 rhs=xt[:, 0:NBLK * BH],
                                 start=True, stop=False)
                nc.tensor.matmul(ps[:, 0:BH], lhsT=cvt[:, 0:128],
                                 rhs=cvt[:, 128:192],
                                 start=False, stop=False)
                nc.tensor.matmul(ps, lhsT=wt[:, 128:256],
                                 rhs=xt[:, BH:(NBLK + 1) * BH],
                                 start=False, stop=True)

                ot = op.tile([128, NBLK * BH], F32, name="ot")
                nc.vector.tensor_copy(ot, ps)

                dst = yap.copy()
                dst.ap = dst.ap[:0] + [[BH, 128], [128 * BH, NBLK], [1, BH]]
                dst.offset = d * T * BH
                nc.gpsimd.dma_start(out=dst, in_=ot)
    nc.compile()
    return nc


def _get_nc(dc: int = DC):
    if dc not in _NC_CACHE:
        _NC_CACHE[dc] = _build_nc(dc)
    return _NC_CACHE[dc]


def _build_filters(l_filter: np.ndarray, r_filter: np.ndarray):
    """Host-side filter prep.

    Returns wmat [128, 256, D] (k, i; W1 = [:, :128], W2 = [:, 128:]) and the
    rank-2 block-0 boundary correction corr [2, 128, D].
    """
    c = l_filter[1:].astype(np.float64)            # (9, D) IIR coeffs
    d = c.shape[1]
    a = np.zeros((258, d))
    a[0] = 1.0
    for n in range(1, 258):
        for k in range(1, min(9, n) + 1):
            a[n] += c[k - 1] * a[n - k]
    q0 = 1.0 + l_filter[0].astype(np.float64)
    q1 = r_filter[0].astype(np.float64)
    q2 = r_filter[1].astype(np.float64)

    # wseq[lag + 129] = combined FIR tap at lag, lag in [-129, 253] (0 < -2)
    wseq = np.zeros((383, d))
    for lag in range(-2, 254):
        t = q2 * a[lag + 2]
        if lag + 1 >= 0:
            t = t + q1 * a[lag + 1]
        if lag >= 0:
            t = t + q0 * a[lag]
        wseq[lag + 129] = t

    kk = np.arange(128)[:, None]
    ii = np.arange(128)[None, :]
    w1 = wseq[ii - kk + 255]                       # (128, 128, D)
    w2 = wseq[ii - kk + 127]
    wmat = np.concatenate([w1, w2], axis=1)        # (128, 256, D)

    i1 = np.arange(128)
    corr = np.stack([-(q1[None, :] * a[i1 + 1] + q2[None, :] * a[i1 + 2]),
                     -(q2[None, :] * a[i1 + 1])], axis=0)   # (2, 128, D)
    return wmat.astype(np.float32), corr.astype(np.float32)


def _make_in_maps(v, l_filter, r_filter, n_cores=N_CORES, dc=DC):
    wmat, corr = _build_filters(l_filter, r_filter)
    vr = np.asarray(v, dtype=np.float32).reshape(BH, T, D)
    in_maps = []
    for cid in range(n_cores):
        sl = slice(cid * dc, (cid + 1) * dc)
        xcore = np.zeros((dc, TPAD, BH), np.float32)
        xcore[:, PADF:PADF + T, :] = vr[:, :, sl].transpose(2, 1, 0)
        cvcore = np.zeros((dc, 2, 192), np.float32)
        cvcore[:, :, 0:128] = corr[:, :, sl].transpose(2, 0, 1)
        cvcore[:, :, 128:192] = vr[:, 0:2, sl].transpose(2, 1, 0)
        in_maps.append({
            "x": np.ascontiguousarray(xcore),
            "wmat": np.ascontiguousarray(wmat[:, :, sl].transpose(2, 0, 1)),
            "cv": cvcore,
        })
    return in_maps


def kernel(v: np.ndarray, l_filter: np.ndarray, r_filter: np.ndarray,
           **_unused) -> np.ndarray:
    nc = _get_nc(DC)
    in_maps = _make_in_maps(v, l_filter, r_filter)
    res = bass_utils.run_bass_kernel_spmd(nc, in_maps, core_ids=list(range(N_CORES)))
    out = np.empty((D, T, BH), np.float32)
    for cid in range(N_CORES):
        out[cid * DC:(cid + 1) * DC] = res.results[cid]["y"]
    return np.ascontiguousarray(out.transpose(2, 1, 0)).reshape(B, H, T, D)
